# revision 7
# baseline (speedup 1.0000x reference)
"""DeltaNet (2-timescale gated linear attention) Trainium2 Bass kernel.

Problem (hardcoded): B=4, T=2048, C=1024, H=8 heads, D=128, K=2 timescales.
  q,k,v = rope(x@Wq), rope(x@Wk), x@Wv ; beta = sigmoid(x@Wb+bb) ;
  mix = softmax(x@Wm+bm) ; scan S_kk = beta_kk*S_kk + k v^T ;
  y_t = sum_kk mix_kk * (q_t @ S_kk) ; out = y@Wo + bo.  (mask == ones)

Sharding: 8 cores = (batch b in 0..3) x (head-group g in 0..1, 4 heads each).
Each core computes a partial out = y_g @ Wo[g-rows] for its (b, g); host sums
the two partials per batch and adds bo.

On-core algorithm: chunked scan, chunk L=256 (2 sub-tiles of 128), with the
two timescales packed side-by-side in the matmul free dim (N=256), fp32r
matmuls throughout (measured ~1.3e-4 relmax per 1024-deep contraction).

RoPE is applied in "rotate-half" layout: Wq/Wk columns are permuted on the
host per head (evens first, odds second) so the on-chip rotation is
  rot(q) = q*C2 + swap_halves(q)*S2n
with C2/S2n host-precomputed [T,128] tables. The head-dim permutation of q,k
cancels in q.k^T and in k v^T (d is contracted), so outputs are unchanged.
"""
import numpy as np

import concourse.bass as bass
import concourse.bacc as bacc
import concourse.mybir as mybir
import concourse.tile as tile
from concourse import bass_utils

N_CORES = 8
B, T, C, H, D, KTS = 4, 2048, 1024, 8, 128, 2
HPC = H // 2          # heads per core (4)
CW = HPC * D          # per-core channel width (512)
SCL = 256             # superchunk length
NSC = T // SCL        # 8 superchunks
NTT = T // 128        # 16 t-tiles

f32 = mybir.dt.float32
f32r = mybir.dt.float32r
AOP = mybir.AluOpType


def _build():
    nc = bacc.Bacc("TRN2", target_bir_lowering=False, debug=False,
                   num_devices=N_CORES)

    # ---- DRAM I/O ----
    x_d = nc.dram_tensor("x", [T, C], f32, kind="ExternalInput")
    wq_d = nc.dram_tensor("wq", [C, CW], f32, kind="ExternalInput")
    wk_d = nc.dram_tensor("wk", [C, CW], f32, kind="ExternalInput")
    wv_d = nc.dram_tensor("wv", [C, CW], f32, kind="ExternalInput")
    wbm_d = nc.dram_tensor("wbm", [C, 16], f32, kind="ExternalInput")
    wo_d = nc.dram_tensor("wo", [CW, C], f32, kind="ExternalInput")
    c2_d = nc.dram_tensor("c2", [T, D], f32, kind="ExternalInput")
    s2n_d = nc.dram_tensor("s2n", [T, D], f32, kind="ExternalInput")
    ident_d = nc.dram_tensor("ident", [128, 128], f32, kind="ExternalInput")
    triu_d = nc.dram_tensor("triu", [128, 128], f32, kind="ExternalInput")
    ntriu_d = nc.dram_tensor("ntriu", [128, 128], f32, kind="ExternalInput")
    mask0_d = nc.dram_tensor("mask0", [128, 256], f32, kind="ExternalInput")
    ones1_d = nc.dram_tensor("ones1", [1, 128], f32, kind="ExternalInput")
    bbm_d = nc.dram_tensor("bbm", [128, 16], f32, kind="ExternalInput")
    zeros_d = nc.dram_tensor("zeros", [128, HPC * 256], f32, kind="ExternalInput")
    pout_d = nc.dram_tensor("pout", [T, C], f32, kind="ExternalOutput")

    with tile.TileContext(nc) as tc:
        with (
            tc.tile_pool(name="singles", bufs=1) as singles,
            tc.tile_pool(name="xpool", bufs=2) as xpool,
            tc.tile_pool(name="xtpool", bufs=4) as xtpool,
            tc.tile_pool(name="qkv", bufs=2) as qkvp,
            tc.tile_pool(name="smalls", bufs=3) as smalls,
            tc.tile_pool(name="ropet", bufs=4) as ropet,
            tc.tile_pool(name="qkt", bufs=4) as qktp,
            tc.tile_pool(name="amp", bufs=4) as amp,
            tc.tile_pool(name="vpp", bufs=4) as vpp,
            tc.tile_pool(name="ycp", bufs=4) as ycp,
            tc.tile_pool(name="bigps", bufs=4, space="PSUM") as bigps,
            tc.tile_pool(name="scanps", bufs=4, space="PSUM") as scanps,
        ):
            # ---- persistent SBUF ----
            wq_sb = singles.tile([128, 8, CW], f32r)
            wk_sb = singles.tile([128, 8, CW], f32r)
            wv_sb = singles.tile([128, 8, CW], f32r)
            wbm_sb = singles.tile([128, 8, 16], f32r)
            wo_sb = singles.tile([128, HPC, C], f32r)
            c2_sb = singles.tile([128, NTT, D], f32)
            s2n_sb = singles.tile([128, NTT, D], f32)
            ident_sb = singles.tile([128, 128], f32r)
            triu_sb = singles.tile([128, 128], f32)
            ntriu_sb = singles.tile([128, 128], f32)
            mask0_sb = singles.tile([128, 256], f32)
            ones1_sb = singles.tile([1, 128], f32)
            bbm_sb = singles.tile([128, 16], f32)
            yt_sb = singles.tile([128, HPC, T], f32r)
            s_sb = singles.tile([128, HPC, 256], f32r)

            nc.sync.dma_start(wq_sb[:], wq_d.ap().rearrange("(kc p) n -> p kc n", p=128).bitcast(f32r))
            nc.sync.dma_start(wk_sb[:], wk_d.ap().rearrange("(kc p) n -> p kc n", p=128).bitcast(f32r))
            nc.sync.dma_start(wv_sb[:], wv_d.ap().rearrange("(kc p) n -> p kc n", p=128).bitcast(f32r))
            nc.sync.dma_start(wbm_sb[:], wbm_d.ap().rearrange("(kc p) n -> p kc n", p=128).bitcast(f32r))
            nc.sync.dma_start(wo_sb[:], wo_d.ap().rearrange("(hc p) n -> p hc n", p=128).bitcast(f32r))
            nc.sync.dma_start(c2_sb[:], c2_d.ap().rearrange("(tt p) d -> p tt d", p=128))
            nc.sync.dma_start(s2n_sb[:], s2n_d.ap().rearrange("(tt p) d -> p tt d", p=128))
            nc.sync.dma_start(ident_sb[:], ident_d.ap().bitcast(f32r))
            nc.sync.dma_start(triu_sb[:], triu_d.ap())
            nc.sync.dma_start(ntriu_sb[:], ntriu_d.ap())
            nc.sync.dma_start(mask0_sb[:], mask0_d.ap())
            nc.sync.dma_start(ones1_sb[:], ones1_d.ap())
            nc.sync.dma_start(bbm_sb[:], bbm_d.ap())
            # state init = 0
            nc.sync.dma_start(s_sb[:], zeros_d.ap().rearrange("p (hc n) -> p hc n", hc=HPC).bitcast(f32r))

            def swap_ap(base):
                """[128,128] AP -> same with 64-col halves swapped."""
                return bass.AP(tensor=base.tensor, offset=base.offset + 64,
                               ap=[base.ap[0], [-64, 2], [1, 64]])

            for sc in range(NSC):
                x_t = xpool.tile([128, 2, C], f32r)
                nc.sync.dma_start(
                    x_t[:],
                    x_d.ap()[sc * SCL:(sc + 1) * SCL, :]
                    .rearrange("(tt p) c -> p tt c", p=128).bitcast(f32r))

                q_sb = qkvp.tile([128, 2, CW], f32r, tag="q_sb")
                k_sb = qkvp.tile([128, 2, CW], f32r, tag="k_sb")
                v_sb = qkvp.tile([128, 2, CW], f32r, tag="v_sb")
                lg_sb = smalls.tile([128, 2, 16], f32, tag="lg")
                l_sb = smalls.tile([128, 2, 8], f32, tag="l")
                pinv_sb = smalls.tile([128, 2, 8], f32, tag="pinv")
                pbar_w = smalls.tile([128, 2, 8], f32, tag="w")
                cdec_sb = smalls.tile([128, 8], f32, tag="cdec")
                tr0 = smalls.tile([1, 8], f32, tag="tr0")
                tr1 = smalls.tile([1, 8], f32, tag="tr1")
                tot_rows = [tr0, tr1]

                for tsub in range(2):
                    ti = sc * 2 + tsub
                    # transpose x -> xT blocks (stationary operands)
                    xt_g = []
                    for grp in range(2):
                        ps_xt = bigps.tile([128, 512], f32r, tag="big")
                        for j in range(4):
                            cc = grp * 4 + j
                            nc.tensor.transpose(
                                ps_xt[:, j * 128:(j + 1) * 128],
                                x_t[:, tsub, cc * 128:(cc + 1) * 128],
                                ident_sb[:])
                        xg = xtpool.tile([128, 4, 128], f32r)
                        nc.scalar.copy(xg[:], ps_xt[:])
                        xt_g.append(xg)

                    # projections
                    ps_q = bigps.tile([128, CW], f32, tag="big")
                    ps_k = bigps.tile([128, CW], f32, tag="big")
                    ps_v = bigps.tile([128, CW], f32, tag="big")
                    ps_lg = scanps.tile([128, 16], f32, tag="scan")
                    for cc in range(8):
                        lhs = xt_g[cc // 4][:, cc % 4, :]
                        st, sp = (cc == 0), (cc == 7)
                        nc.tensor.matmul(ps_q[:], lhs, wq_sb[:, cc, :], start=st, stop=sp)
                        nc.tensor.matmul(ps_k[:], lhs, wk_sb[:, cc, :], start=st, stop=sp)
                        nc.tensor.matmul(ps_v[:], lhs, wv_sb[:, cc, :], start=st, stop=sp)
                        nc.tensor.matmul(ps_lg[:], lhs, wbm_sb[:, cc, :], start=st, stop=sp)

                    # RoPE: rot = psum*C2 + swap(psum)*S2n   (per head)
                    for hh in range(HPC):
                        hs = slice(hh * 128, (hh + 1) * 128)
                        for name, ps, dst in (("q", ps_q, q_sb), ("k", ps_k, k_sb)):
                            ra = ropet.tile([128, 128], f32, tag="ra")
                            rb = ropet.tile([128, 128], f32, tag="rb")
                            nc.vector.tensor_mul(ra[:], ps[:, hs], c2_sb[:, ti, :])
                            nc.vector.tensor_mul(rb[:], swap_ap(ps[:, hs]), s2n_sb[:, ti, :])
                            nc.gpsimd.tensor_add(dst[:, tsub, hs], ra[:], rb[:])

                    # v evacuation + logits bias
                    nc.scalar.copy(v_sb[:, tsub, :], ps_v[:])
                    nc.vector.tensor_add(lg_sb[:, tsub, :], ps_lg[:], bbm_sb[:])

                    # -log(sigmoid(blg)) = ln(1 + exp(-blg))  (Exp/Ln table only)
                    e0_t = smalls.tile([128, 8], f32, tag="e0")
                    nc.scalar.activation(e0_t[:], lg_sb[:, tsub, 0:8],
                                         mybir.ActivationFunctionType.Exp, scale=-1.0)
                    lb_t = smalls.tile([128, 8], f32, tag="lb")  # = -log beta
                    nc.scalar.activation(lb_t[:], e0_t[:],
                                         mybir.ActivationFunctionType.Ln, bias=1.0)

                    # inclusive cumsum (negated lhsT restores the sign)
                    ps_l = scanps.tile([128, 8], f32, tag="scan")
                    nc.tensor.matmul(ps_l[:], ntriu_sb[:], lb_t[:], start=True, stop=True)
                    # subtile total via (-1)-column matmul -> [1, 8] at partition 0
                    ps_tt = scanps.tile([128, 8], f32, tag="scan")
                    nc.tensor.matmul(ps_tt[0:1, :], ntriu_sb[:, 127:128], lb_t[:],
                                     start=True, stop=True)
                    nc.scalar.copy(tot_rows[tsub][:], ps_tt[0:1, :])
                    if tsub == 0:
                        nc.scalar.copy(l_sb[:, 0, :], ps_l[:])
                    else:
                        # add carry (total of sub0) broadcast over partitions
                        ps_c = scanps.tile([128, 8], f32, tag="scan")
                        nc.tensor.matmul(ps_c[:], ones1_sb[:], tot_rows[0][:],
                                         start=True, stop=True)
                        cb0 = smalls.tile([128, 8], f32, tag="cb0")
                        nc.scalar.copy(cb0[:], ps_c[:])
                        nc.vector.tensor_add(l_sb[:, 1, :], ps_l[:], cb0[:])

                # chunk-level decay quantities
                tot_all = smalls.tile([1, 8], f32, tag="tot_all")
                nc.vector.tensor_add(tot_all[:], tot_rows[0][:], tot_rows[1][:])
                cbc = smalls.tile([128, 8], f32, tag="cbc")
                ps_c2 = scanps.tile([128, 8], f32, tag="scan")
                nc.tensor.matmul(ps_c2[:], ones1_sb[:], tot_all[:],
                                 start=True, stop=True)
                nc.scalar.copy(cbc[:], ps_c2[:])
                nc.scalar.activation(cdec_sb[:], cbc[:], mybir.ActivationFunctionType.Exp)
                for tsub in range(2):
                    nc.scalar.activation(pinv_sb[:, tsub, :], l_sb[:, tsub, :],
                                         mybir.ActivationFunctionType.Exp, scale=-1.0)
                    # w = mix * exp(L); mix0 = sig(m0-m1), mix1 = sig(m1-m0)
                    md = smalls.tile([128, 4], f32, tag="md")
                    nc.vector.tensor_sub(
                        md[:],
                        lg_sb[:, tsub, 8:16].rearrange("p (h two) -> p h two", two=2)[:, :, 0],
                        lg_sb[:, tsub, 8:16].rearrange("p (h two) -> p h two", two=2)[:, :, 1])
                    mixt = smalls.tile([128, 8], f32, tag="mixt")
                    mixv = mixt[:].rearrange("p (h two) -> p h two", two=2)
                    e1_t = smalls.tile([128, 4], f32, tag="e1")
                    nc.scalar.activation(e1_t[:], md[:],
                                         mybir.ActivationFunctionType.Exp, scale=-1.0)
                    r1_t = smalls.tile([128, 4], f32, tag="r1")
                    nc.vector.tensor_scalar_add(r1_t[:], e1_t[:], 1.0)
                    nc.vector.reciprocal(mixv[:, :, 0], r1_t[:])
                    nc.vector.tensor_mul(mixv[:, :, 1], e1_t[:], mixv[:, :, 0])
                    pt = smalls.tile([128, 8], f32, tag="pt")
                    nc.scalar.activation(pt[:], l_sb[:, tsub, :],
                                         mybir.ActivationFunctionType.Exp)
                    nc.vector.tensor_mul(pbar_w[:, tsub, :], mixt[:], pt[:])

                # ---- scan per head ----
                for hh in range(HPC):
                    hs = slice(hh * 128, (hh + 1) * 128)
                    # qT, kT via PE transpose
                    ps_qt = scanps.tile([128, 256], f32r, tag="scan")
                    nc.tensor.transpose(ps_qt[:, 0:128], q_sb[:, 0, hs], ident_sb[:])
                    nc.tensor.transpose(ps_qt[:, 128:256], q_sb[:, 1, hs], ident_sb[:])
                    qt = qktp.tile([128, 256], f32r, tag="qt")
                    nc.scalar.copy(qt[:], ps_qt[:])
                    ps_kt = scanps.tile([128, 256], f32r, tag="scan")
                    nc.tensor.transpose(ps_kt[:, 0:128], k_sb[:, 0, hs], ident_sb[:])
                    nc.tensor.transpose(ps_kt[:, 128:256], k_sb[:, 1, hs], ident_sb[:])
                    kt = qktp.tile([128, 256], f32r, tag="kt")
                    nc.scalar.copy(kt[:], ps_kt[:])

                    # A = (kT)^T qT in [s, t] coords
                    ps_a0 = scanps.tile([128, 256], f32, tag="scan")
                    nc.tensor.matmul(ps_a0[:], kt[:, 0:128], qt[:], start=True, stop=True)
                    ps_a1 = scanps.tile([128, 256], f32, tag="scan")
                    nc.tensor.matmul(ps_a1[:], kt[:, 128:256], qt[:], start=True, stop=True)
                    am0 = amp.tile([128, 256], f32r, tag="am0")
                    nc.vector.tensor_mul(am0[:], ps_a0[:], mask0_sb[:])
                    am1 = amp.tile([128, 128], f32r, tag="am1")
                    nc.vector.tensor_mul(am1[:], ps_a1[:, 128:256], triu_sb[:])

                    # V' pairs: [Pinv_kk0 * v | Pinv_kk1 * v]
                    vp = []
                    for ss in range(2):
                        vpt = vpp.tile([128, 256], f32r, tag="vp")
                        for kk in range(2):
                            nc.vector.tensor_scalar_mul(
                                vpt[:, kk * 128:(kk + 1) * 128],
                                v_sb[:, ss, hs],
                                pinv_sb[:, ss, 2 * hh + kk:2 * hh + kk + 1].opt())
                        vp.append(vpt)

                    # y(t0), y(t1) with timescales packed in N
                    ps_y0 = scanps.tile([128, 256], f32, tag="scan")
                    nc.tensor.matmul(ps_y0[:], am0[:, 0:128], vp[0][:], start=True, stop=False)
                    nc.tensor.matmul(ps_y0[:], qt[:, 0:128], s_sb[:, hh, :], start=False, stop=True)
                    ps_y1 = scanps.tile([128, 256], f32, tag="scan")
                    nc.tensor.matmul(ps_y1[:], am0[:, 128:256], vp[0][:], start=True, stop=False)
                    nc.tensor.matmul(ps_y1[:], am1[:], vp[1][:], start=False, stop=False)
                    nc.tensor.matmul(ps_y1[:], qt[:, 128:256], s_sb[:, hh, :], start=False, stop=True)

                    # state update: S = cdec * (S + k^T V')
                    ps_s = scanps.tile([128, 256], f32, tag="scan")
                    nc.tensor.matmul(ps_s[:], k_sb[:, 0, hs], vp[0][:], start=True, stop=False)
                    nc.tensor.matmul(ps_s[:], k_sb[:, 1, hs], vp[1][:], start=False, stop=True)
                    nc.vector.tensor_add(ps_s[:], ps_s[:], s_sb[:, hh, :].bitcast(f32))
                    for kk in range(2):
                        nc.vector.tensor_scalar_mul(
                            s_sb[:, hh, kk * 128:(kk + 1) * 128],
                            ps_s[:, kk * 128:(kk + 1) * 128],
                            cdec_sb[:, 2 * hh + kk:2 * hh + kk + 1].opt())

                    # combine timescales: y = w0*y_kk0 + w1*y_kk1, then yT
                    ps_yt = scanps.tile([128, 256], f32r, tag="scan")
                    for tsub, ps_y in ((0, ps_y0), (1, ps_y1)):
                        ytmp = ycp.tile([128, 128], f32, tag="ytmp")
                        nc.vector.tensor_scalar_mul(
                            ytmp[:], ps_y[:, 0:128],
                            pbar_w[:, tsub, 2 * hh:2 * hh + 1].opt())
                        yc = ycp.tile([128, 128], f32r, tag="yc")
                        nc.vector.scalar_tensor_tensor(
                            yc[:], ps_y[:, 128:256],
                            pbar_w[:, tsub, 2 * hh + 1:2 * hh + 2].opt(),
                            ytmp[:], op0=AOP.mult, op1=AOP.add)
                        nc.tensor.transpose(ps_yt[:, tsub * 128:(tsub + 1) * 128],
                                            yc[:], ident_sb[:])
                    nc.scalar.copy(yt_sb[:, hh, sc * 256:(sc + 1) * 256], ps_yt[:])

            # ---- output projection: pout[t, :] = sum_h yT_h^T @ Wo_h ----
            for tt in range(NTT):
                for co in range(2):
                    ps_o = bigps.tile([128, 512], f32, tag="big")
                    for hh in range(HPC):
                        nc.tensor.matmul(
                            ps_o[:],
                            yt_sb[:, hh, tt * 128:(tt + 1) * 128],
                            wo_sb[:, hh, co * 512:(co + 1) * 512],
                            start=(hh == 0), stop=(hh == HPC - 1))
                    ost = ycp.tile([128, 512], f32, tag="ost")
                    nc.scalar.copy(ost[:], ps_o[:])
                    nc.sync.dma_start(
                        pout_d.ap()[tt * 128:(tt + 1) * 128, co * 512:(co + 1) * 512],
                        ost[:])

    nc.compile()
    return nc


_CACHE = {}


def _get_nc():
    if "nc" not in _CACHE:
        _CACHE["nc"] = _build()
    return _CACHE["nc"]


def _host_consts():
    if "consts" in _CACHE:
        return _CACHE["consts"]
    inv_freq = 1.0 / (10000.0 ** (np.arange(0, D, 2, dtype=np.float32) / D))
    tt = np.arange(T, dtype=np.float32)[:, None]
    cos = np.cos(tt * inv_freq[None, :]).astype(np.float32)
    sin = np.sin(tt * inv_freq[None, :]).astype(np.float32)
    c2 = np.concatenate([cos, cos], axis=1)
    s2n = np.concatenate([-sin, sin], axis=1)
    ident = np.eye(128, dtype=np.float32)
    triu = np.triu(np.ones((128, 128), np.float32))
    mask0 = np.concatenate([triu, np.ones((128, 128), np.float32)], axis=1)
    ntriu = -triu
    ones1 = np.ones((1, 128), np.float32)
    zeros = np.zeros((128, HPC * 256), np.float32)
    perm = np.zeros(C, dtype=np.int64)
    for h in range(H):
        base = h * D
        perm[base:base + D // 2] = base + 2 * np.arange(D // 2)
        perm[base + D // 2:base + D] = base + 2 * np.arange(D // 2) + 1
    _CACHE["consts"] = (c2, s2n, ident, triu, ntriu, mask0, ones1, zeros, perm)
    return _CACHE["consts"]


def _kernel_numpy_fallback(x, mask, Wq, Wk, Wv, Wb, bb, Wm, bm, Wo, bo):
    """Exact-semantics host fallback (only used if mask is not all ones)."""
    out = np.zeros((B, T, C), np.float32)
    for b in range(B):
        xb = x[b]
        q = (xb @ Wq).reshape(T, H, D)
        k = (xb @ Wk).reshape(T, H, D)
        v = (xb @ Wv).reshape(T, H, D)
        inv_freq = 1.0 / (10000.0 ** (np.arange(0, D, 2, dtype=np.float32) / D))
        fr = np.arange(T, dtype=np.float32)[:, None] * inv_freq[None, :]
        cos, sin = np.cos(fr), np.sin(fr)

        def rope(z):
            zp = z.reshape(T, H, D // 2, 2)
            zr, zi = zp[..., 0], zp[..., 1]
            rr = zr * cos[:, None] - zi * sin[:, None]
            ri = zr * sin[:, None] + zi * cos[:, None]
            return np.stack([rr, ri], -1).reshape(T, H, D)
        q, k = rope(q), rope(k)
        beta = 1 / (1 + np.exp(-(xb @ Wb + bb))).reshape(T, H, KTS)
        ml = (xb @ Wm + bm).reshape(T, H, KTS)
        ml = ml - ml.max(-1, keepdims=True)
        mx = np.exp(ml)
        mx /= mx.sum(-1, keepdims=True)
        S = np.zeros((H, KTS, D, D), np.float32)
        for t in range(T):
            if mask[b, t] > 0:
                S = beta[t][:, :, None, None] * S + \
                    np.einsum('hd,he->hde', k[t], v[t])[:, None]
                yk = np.einsum('hd,hkde->hke', q[t], S)
                out[b, t] = (yk * mx[t][:, :, None]).sum(1).reshape(C)
    return out @ Wo + bo


def kernel(x, mask, Wq, Wk, Wv, Wb, bb, Wm, bm, Wo, bo):
    x = np.ascontiguousarray(np.asarray(x, np.float32))
    mask = np.asarray(mask)
    if not np.all(mask == 1):
        return (_kernel_numpy_fallback(
            x, mask, *(np.asarray(a, np.float32) for a in
                       (Wq, Wk, Wv, Wb, bb, Wm, bm, Wo, bo)))).astype(np.float32)

    Wq, Wk, Wv, Wb, Wm, Wo = (np.asarray(a, np.float32) for a in (Wq, Wk, Wv, Wb, Wm, Wo))
    bb, bm, bo = (np.asarray(a, np.float32) for a in (bb, bm, bo))
    c2, s2n, ident, triu, ntriu, mask0, ones1, zeros, perm = _host_consts()
    Wq_p, Wk_p = Wq[:, perm], Wk[:, perm]

    nc = _get_nc()
    in_maps = []
    for core in range(N_CORES):
        b, g = core // 2, core % 2
        cs = slice(g * CW, (g + 1) * CW)
        us = slice(g * 8, (g + 1) * 8)
        bbm = np.tile(np.concatenate([bb[us], bm[us]])[None, :], (128, 1)).astype(np.float32)
        in_maps.append({
            "x": np.ascontiguousarray(x[b]),
            "wq": np.ascontiguousarray(Wq_p[:, cs]),
            "wk": np.ascontiguousarray(Wk_p[:, cs]),
            "wv": np.ascontiguousarray(Wv[:, cs]),
            "wbm": np.ascontiguousarray(np.concatenate([Wb[:, us], Wm[:, us]], axis=1)),
            "wo": np.ascontiguousarray(Wo[cs, :]),
            "c2": c2, "s2n": s2n, "ident": ident, "triu": triu,
            "ntriu": ntriu,
            "mask0": mask0, "ones1": ones1, "bbm": bbm, "zeros": zeros,
        })
    res = bass_utils.run_bass_kernel_spmd(nc, in_maps, core_ids=list(range(N_CORES)))
    out = np.empty((B, T, C), np.float32)
    for b in range(B):
        out[b] = res.results[2 * b]["pout"] + res.results[2 * b + 1]["pout"]
    out += bo[None, None, :]
    return out
